# revision 1
# baseline (speedup 1.0000x reference)
"""BinaryXnorExceptOutliersLinear forward on 8 TRN2 NeuronCores.

out = x @ w_sim.T + bias, where w_sim binarizes non-outlier weights to
sign(w) * mean(|w| over non-outliers) and keeps outliers (|w - mean| >
1.6 * std, global scalar stats) at full precision.

Strategy (column-parallel / tensor-parallel on out_features):
  - host: transpose x -> xT [4096, 8192] (replicated to all cores) and
    weight -> wT [4096, 4096], shard wT / bias along out_features (512/core).
  - device: global weight stats via two tiny AllReduces (sum/sumsq, then
    masked |w| sum + count), binarize the local shard in SBUF, then a dense
    fp32r matmul (full TensorE rate at N=512, ~13 mantissa bits) streaming
    xT k-slices; bias added during PSUM->SBUF eviction on ScalarE.
  - host: concatenate the per-core [512, 8192] outT shards, transpose back.
"""

import numpy as np

import concourse.bass as bass
import concourse.mybir as mybir
from concourse.alu_op_type import AluOpType
from concourse.bass_utils import run_bass_kernel_spmd
from concourse.vector_clock import ScopedClock

import bass_rust
import concourse.tile as tile

F = mybir.ActivationFunctionType
FP32 = mybir.dt.float32
FP32R = mybir.dt.float32r
U8 = mybir.dt.uint8
X = mybir.AxisListType.X

N_CORES = 8
D_IN = 4096
D_OUT = 4096
TOK = 8192            # 4 * 2048 tokens
D_OUT_SH = D_OUT // N_CORES   # 512 out features per core
KC = D_IN // 128      # 32 k-chunks
MSUB = D_OUT_SH // 128  # 4 psum-partition chunks of out features
TOK_TILE = 512
N_TOKT = TOK // TOK_TILE  # 16
N_ELEM = D_OUT * D_IN     # full-weight element count for global stats
STD_K = 1.6


class _LegalTileContext(tile.TileContext):
    """TileContext that legalizes sem waits for this walrus build.

    The walrus here encodes a single wait slot per 64B instruction, so any
    instruction Tile annotates with N>1 sem waits fails codegen ("Too many
    sync wait commands").  Split the extras onto single-wait NOPs placed
    immediately before the instruction on the same engine, and do the same
    for the exit drain's global-clock waits.
    """

    def _add_instruction(self, inst):
        si = inst.sync_info
        if si is not None and si.on_wait and len(si.on_wait) > 1:
            waits = list(si.on_wait)
            for w in waits[:-1]:
                nop = bass_rust.InstNoOp(
                    text_hint="wait_split",
                    bass_nofuse=True,
                    name=self.nc.get_next_instruction_name(),
                    engine=inst.engine,
                    sync_info=mybir.SyncInfo(on_wait=[w], on_update=[]),
                )
                super()._add_instruction(nop)
            si.on_wait = waits[-1:]
            inst.sync_info = si
        super()._add_instruction(inst)

    def _drain_and_barrier(self, tick_clock, wait_clock):
        probe = self.nc.sync.nop(hint="drain_wait_probe", nofuse=True)
        wait_clock.add_sem_waits(
            probe.ins, ScopedClock({None: tick_clock.global_clock})
        )
        waits = list(probe.ins.sync_info.on_wait or []) if probe.ins.sync_info else []
        if len(waits) > 1:
            probe.ins.sync_info.on_wait = waits[:1]
            for w in waits[1:]:
                nop = self.nc.sync.nop(hint="drain_wait_split", nofuse=True)
                si = nop.ins.sync_info
                if si is None:
                    nop.ins.sync_info = mybir.SyncInfo(on_wait=[w], on_update=[])
                else:
                    si.on_wait = [w]
        self.nc.sync.drain()
        self.nc.all_engine_barrier()
        assert self.sems is not None
        popped = self.nc._tile_sem_poison_stack.pop()
        assert popped is self._sem_poison
        self.nc.clear_and_free_semaphores(list(self.sems.allocated().values()))
        self.nc.all_engine_barrier()


def _build_program():
    nc = bass.Bass()
    xt_in = nc.dram_tensor("xt", [D_IN, TOK], FP32, kind="ExternalInput")
    wt_in = nc.dram_tensor("wt", [D_IN, D_OUT_SH], FP32, kind="ExternalInput")
    b_in = nc.dram_tensor("bias", [128, MSUB], FP32, kind="ExternalInput")
    out_t = nc.dram_tensor("out", [D_OUT_SH, TOK], FP32, kind="ExternalOutput")

    with _LegalTileContext(nc) as tc:
        with (
            tc.tile_pool(name="wsim", bufs=1) as wsim_p,
            tc.tile_pool(name="consts", bufs=1) as cp,
            tc.tile_pool(name="stats", bufs=1) as st,
            tc.tile_pool(name="dram", bufs=1, space="DRAM") as dram,
        ):
            # ---- collective firmware warmup (no data deps) ----------------
            warm_i = dram.tile([1, 1], FP32)
            warm_o = dram.tile([1, 1], FP32)
            nc.gpsimd.dma_start(warm_i[:], b_in[0:1, 0:1])
            nc.gpsimd.collective_compute(
                "AllReduce", mybir.AluOpType.add,
                replica_groups=[list(range(N_CORES))],
                ins=[warm_i.opt()], outs=[warm_o.opt()],
            )

            # ---- constants -------------------------------------------------
            ones_col = cp.tile([128, 1], FP32)
            nc.vector.memset(ones_col[:], 1.0)
            ones_row = cp.tile([1, 128], FP32)
            nc.vector.memset(ones_row[:], 1.0)
            bias_sb = cp.tile([128, MSUB], FP32)
            nc.sync.dma_start(bias_sb[:], b_in[:])

            gstats = st.tile([1, 12], FP32)
            wsim = [wsim_p.tile([128, D_OUT_SH], FP32R, name=f"wsim{k}", tag=f"wsim{k}")
                    for k in range(KC)]

            xs_cm = tc.tile_pool(name="xs", bufs=11)
            xp = xs_cm.__enter__()
            outs_cm = tc.tile_pool(name="outs", bufs=6)
            op = outs_cm.__enter__()

            with (
                tc.tile_pool(name="wraw", bufs=1) as wp,
                tc.tile_pool(name="masks", bufs=1) as mp,
                tc.tile_pool(name="scr", bufs=2) as sp,
            ):
                ps_s_cm = tc.tile_pool(name="psum_s", bufs=1, space="PSUM")
                ps_s = ps_s_cm.__enter__()
                wt = []
                for k in range(KC):
                    t = wp.tile([128, D_OUT_SH], FP32, tag=f"w{k}")
                    nc.sync.dma_start(t[:], wt_in[k * 128:(k + 1) * 128, :])
                    wt.append(t)

                # ---- phase A1: global sum / sumsq / sum|w|; sign mask -----
                acc = st.tile([128, 2 * KC], FP32)
                sum_ps = [ps_s.tile([128, 1], FP32, name=f"sums{m}", tag=f"sums{m}")
                          for m in range(MSUB)]
                for k in range(KC):
                    for m in range(MSUB):
                        nc.tensor.matmul(sum_ps[m][:],
                                         wt[k][:, m * 128:(m + 1) * 128],
                                         ones_col[:],
                                         start=(k == 0), stop=(k == KC - 1))
                    sq2 = sp.tile([128, D_OUT_SH], FP32, tag="scrA")
                    nc.scalar.activation(sq2[:], wt[k][:], F.Square,
                                         accum_out=acc[:, 2 * k:2 * k + 1])
                    nc.vector.reduce_sum(acc[:, 2 * k + 1:2 * k + 2], wt[k][:], axis=X,
                                         apply_absolute_value=True)
                acc2 = st.tile([128, 3], FP32)
                sums_sb = st.tile([128, MSUB], FP32)
                for m in range(MSUB):
                    nc.vector.tensor_copy(sums_sb[:, m:m + 1], sum_ps[m][:])
                nc.vector.reduce_sum(acc2[:, 0:1], sums_sb[:], axis=X)
                nc.vector.reduce_sum(acc2[:, 1:2], acc[:, 0::2], axis=X)
                nc.vector.reduce_sum(acc2[:, 2:3], acc[:, 1::2], axis=X)
                p1 = ps_s.tile([1, 3], FP32)
                nc.tensor.matmul(p1[:], ones_col[:], acc2[:], start=True, stop=True)

                bnc1 = dram.tile([1, 3], FP32)
                bnc1o = dram.tile([1, 3], FP32)
                sb1 = st.tile([1, 3], FP32)
                nc.vector.tensor_copy(sb1[:], p1[:])
                nc.gpsimd.dma_start(bnc1[:], sb1[:])
                nc.gpsimd.collective_compute(
                    "AllReduce", mybir.AluOpType.add,
                    replica_groups=[list(range(N_CORES))],
                    ins=[bnc1.opt()], outs=[bnc1o.opt()],
                )
                nc.gpsimd.dma_start(gstats[:, 0:3], bnc1o[:])

                # ---- global scalar math: mean, thr ------------------------
                S = gstats[:, 0:1]; SS = gstats[:, 1:2]
                mean = gstats[:, 3:4]; thr = gstats[:, 4:5]
                var = gstats[:, 5:6]
                lower = gstats[:, 8:9]; upper = gstats[:, 9:10]
                nc.scalar.mul(mean, S, 1.0 / N_ELEM)
                nc.vector.tensor_mul(var, S, mean)
                nc.vector.tensor_sub(var, SS, var)
                nc.scalar.mul(var, var, 1.0 / (N_ELEM - 1))
                nc.scalar.sqrt(var, var)
                nc.scalar.mul(thr, var, STD_K)
                nc.vector.tensor_sub(lower, mean, thr)
                nc.vector.tensor_add(upper, mean, thr)

                # broadcast [lower, upper] to all partitions via ones-row matmul
                pb = ps_s.tile([128, 2], FP32)
                nc.tensor.matmul(pb[:], ones_row[:], gstats[0:1, 8:10], start=True, stop=True)
                blu = cp.tile([128, 2], FP32)
                nc.vector.tensor_copy(blu[:], pb[:])

                # ---- phase A2: outlier masks + masked sums ---------------
                macc = st.tile([128, 2 * KC], FP32)
                om = []
                for k in range(KC):
                    hi = sp.tile([128, D_OUT_SH], FP32, tag="scrA")
                    nc.vector.tensor_scalar(hi[:], wt[k][:], blu[:, 1:2], None,
                                            op0=AluOpType.is_gt)
                    m = mp.tile([128, D_OUT_SH], U8, name=f"om{k}", tag=f"om{k}")
                    nc.vector.scalar_tensor_tensor(
                        m[:], wt[k][:], blu[:, 0:1], hi[:],
                        AluOpType.is_lt, AluOpType.logical_or,
                        accum_out=macc[:, 2 * k:2 * k + 1])
                    om.append(m)
                    absw = sp.tile([128, D_OUT_SH], FP32, tag="scrB")
                    nc.scalar.activation(absw[:], wt[k][:], F.Abs)
                    junk = sp.tile([128, D_OUT_SH], FP32, tag="scrC")
                    nc.vector.scalar_tensor_tensor(
                        junk[:], absw[:], 1.0, m[:],
                        AluOpType.mult, AluOpType.mult,
                        accum_out=macc[:, 2 * k + 1:2 * k + 2])
                macc2 = st.tile([128, 2], FP32)
                for j in range(2):
                    nc.vector.reduce_sum(macc2[:, j:j + 1], macc[:, j::2], axis=X)
                p2 = ps_s.tile([1, 2], FP32)
                nc.tensor.matmul(p2[:], ones_col[:], macc2[:], start=True, stop=True)

                bnc2 = dram.tile([1, 2], FP32)
                bnc2o = dram.tile([1, 2], FP32)
                sb2 = st.tile([1, 2], FP32)
                nc.vector.tensor_copy(sb2[:], p2[:])
                nc.gpsimd.dma_start(bnc2[:], sb2[:])
                nc.gpsimd.collective_compute(
                    "AllReduce", mybir.AluOpType.add,
                    replica_groups=[list(range(N_CORES))],
                    ins=[bnc2.opt()], outs=[bnc2o.opt()],
                )
                nc.gpsimd.dma_start(gstats[:, 6:8], bnc2o[:])

                # binary_scale = (sum|w| - sum|w|*out) / (N - count_out)
                sabs = gstats[:, 2:3]; cnto = gstats[:, 6:7]; sabso = gstats[:, 7:8]
                num = gstats[:, 8:9]; den = gstats[:, 9:10]; scl = gstats[:, 10:11]
                nc.vector.tensor_sub(num, sabs, sabso)
                nc.vector.tensor_scalar(den, cnto, -1.0, float(N_ELEM),
                                        op0=AluOpType.mult, op1=AluOpType.add)
                nc.vector.reciprocal(den, den)
                nc.vector.tensor_mul(scl, num, den)
                # broadcast scale and -scale
                sc2 = gstats[:, 10:12]
                nc.scalar.mul(gstats[:, 11:12], scl, -1.0)
                pb2 = ps_s.tile([128, 2], FP32)
                nc.tensor.matmul(pb2[:], ones_row[:], sc2, start=True, stop=True)
                bscale = cp.tile([128, 2], FP32)
                nc.vector.tensor_copy(bscale[:], pb2[:])
                ps_s_cm.__exit__(None, None, None)

                # ---- phase B: build w_sim (fp32r) -------------------------
                # bin = sgn_mask * 2*scale - scale  (sgn_mask in {0,1})
                # wsim = bin + om * (w - bin)
                two_scale = cp.tile([128, 1], FP32)
                nc.scalar.mul(two_scale[:], bscale[:, 0:1], 2.0)
                for k in range(KC):
                    sm = sp.tile([128, D_OUT_SH], U8, tag="scrS")
                    nc.vector.tensor_scalar(sm[:], wt[k][:], 0.0, None,
                                            op0=AluOpType.is_ge)
                    bin_f = sp.tile([128, D_OUT_SH], FP32, tag="scrA")
                    nc.scalar.activation(bin_f[:], sm[:], F.Identity,
                                         scale=two_scale[:, 0:1],
                                         bias=bscale[:, 1:2])
                    dlt = sp.tile([128, D_OUT_SH], FP32, tag="scrB")
                    nc.vector.tensor_sub(dlt[:], wt[k][:], bin_f[:])
                    nc.vector.tensor_tensor(dlt[:], dlt[:], om[k][:], op=AluOpType.mult)
                    nc.vector.tensor_tensor(wsim[k][:], bin_f[:], dlt[:], op=AluOpType.add)

            # ---- phase C: dense matmul ------------------------------------
            with (
                tc.tile_pool(name="ops", bufs=2, space="PSUM") as pp,
            ):
                for tt in range(N_TOKT):
                    t0 = tt * TOK_TILE
                    psum = [pp.tile([128, TOK_TILE], FP32, name=f"ps_{tt}_{m}",
                                    tag=f"ps{m}")
                            for m in range(MSUB)]
                    for k in range(KC):
                        xt_t = xp.tile([128, TOK_TILE], FP32R, tag="xt")
                        nc.sync.dma_start(
                            xt_t[:],
                            xt_in[k * 128:(k + 1) * 128, t0:t0 + TOK_TILE].bitcast(FP32R))
                        for m in range(MSUB):
                            nc.tensor.matmul(
                                psum[m][:],
                                wsim[k][:, m * 128:(m + 1) * 128],
                                xt_t[:],
                                start=(k == 0), stop=(k == KC - 1))
                    for m in range(MSUB):
                        ot = op.tile([128, TOK_TILE], FP32, name=f"ot_{tt}_{m}",
                                     tag="ot")
                        nc.scalar.activation(ot[:], psum[m][:], F.Identity,
                                             bias=bias_sb[:, m:m + 1])
                        nc.gpsimd.dma_start(
                            out_t[m * 128:(m + 1) * 128, t0:t0 + TOK_TILE], ot[:])
            outs_cm.__exit__(None, None, None)
            xs_cm.__exit__(None, None, None)
    return nc


_NC_CACHE = None


def _get_program():
    global _NC_CACHE
    if _NC_CACHE is None:
        _NC_CACHE = _build_program()
    return _NC_CACHE


def _make_in_maps(x, weight, bias):
    xT = np.ascontiguousarray(x.reshape(TOK, D_IN).T)  # [D_IN, TOK]
    in_maps = []
    for c in range(N_CORES):
        o0 = c * D_OUT_SH
        wT_c = np.ascontiguousarray(weight[o0:o0 + D_OUT_SH, :].T)  # [D_IN, 512]
        b_c = np.ascontiguousarray(
            bias[o0:o0 + D_OUT_SH].reshape(MSUB, 128).T)  # [128, MSUB]
        in_maps.append({"xt": xT, "wt": wT_c, "bias": b_c})
    return in_maps


def kernel(x: np.ndarray, weight: np.ndarray, bias: np.ndarray) -> np.ndarray:
    nc = _get_program()
    in_maps = _make_in_maps(x, weight, bias)
    res = run_bass_kernel_spmd(nc, in_maps, list(range(N_CORES)))
    outT = np.concatenate([res.results[c]["out"] for c in range(N_CORES)], axis=0)
    return np.ascontiguousarray(outT.T).reshape(x.shape[0], x.shape[1], D_OUT)

